# revision 14
# baseline (speedup 1.0000x reference)
"""Trainium2 Bass kernel for nn_DirectionalWeights (GNN edge softmax).

Math (reference):
  a1 = LN(nf @ W1) * g1 ;  a2 = LN(nf @ W2) * g2     (b/bb are zero)
  Zij = relu(a1[s] + a2[t]) @ W3 + b3 ;  Zji = relu(a1[t] + a2[s]) @ W3 + b3
  d = Zij - Zji ; Vij = relu(w4*d + b4) ; Vji = relu(-w4*d + b4)
  out_ij = segment_softmax(Vij by src) ; out_ji = segment_softmax(Vji by dst)

With w = W3[:,0] and X_m = w * a_m (w folded, signed), columns permuted
pos-w-first (posl pos cols, nneg neg cols):
  d = PLUS - MINUS + delta(i) - delta(j)
  PLUS  = sum_pos relu(X1p(i)+X2p(j)) + sum_neg relu(X2n(i)+X1n(j))
  MINUS = sum_neg relu(X1n(i)+X2n(j)) + sum_pos relu(X2p(i)+X1p(j))
  delta(n) = sum_neg X1[n] - sum_neg X2[n]
Row layouts (512 = posl+nneg):
  res1 row (SBUF, per owned node i): [X1p, X2n | X2p, X1n]   (PLUS | MINUS in0)
  Y row (DRAM, gathered for node j): [X2p, X1n | X1p, X2n | delta, 0pad]
So the edge kernel is TWO 512-wide fused DVE ops per edge (relu-add-reduce
with imm=+/-1), one delta(j) column fix per gather call, one delta(i)
per-tile scalar add.  d is computed ONCE per edge (single pass grouped by
src); the dst-grouped softmax gets d via a tiny bf16 d-table AllGather and
a 256B-row dma_gather + one-hot select:
  S = sum_k relu(dtab[k] + M[k]),  M = +64 at the edge's lane, -64 else
  => S = d + 64 ;  Vji = relu(-w4*S + (b4 + 64*w4))

Sharding: batch b = core//4, node-quarter q = core%4. Collectives use
Shared scratchpad outputs (fast ncfw path).
"""

import numpy as np
import ml_dtypes

import concourse.bass as bass
import concourse.mybir as mybir
import concourse.tile as tile
from concourse import library_config
from concourse.bass_utils import run_bass_kernel_spmd

# ---------------------------------------------------------------- constants
B, N, E, F, H = 2, 10000, 100000, 512, 512
EPS = 1e-5
NQ = 4              # node quarters (cores per batch)
NSH = 2560          # padded nodes per shard (20 tiles of 128)
NT = NSH // 128     # node tiles per shard
BW = 512            # block width (posl + nneg)
ROW = 1280          # Y row (fp8): [PLUS 512 | MINUS 512 | delta, 255 pad]
BIGS = 64.0         # select offset for the d-table one-hot trick
NCHUNK = 4          # Y allgather chunks
MAXSLOT = 8         # max slots per dma_gather call
LSW = 12            # local_scatter bucket depth per (src-p, dst-p) pair
bf16 = mybir.dt.bfloat16
f8 = mybir.dt.float8e4
SCL = 16.0          # fp8 dynamic-range scale folded into G
f32 = mybir.dt.float32

_WAITFIX_MAX = 1


def _split_waits(nc, max_waits=_WAITFIX_MAX):
    """This walrus build rejects >1 sync wait per instruction; hoist excess
    waits onto inserted same-engine NoOps."""
    from bass_rust import InstNoOp

    ctr = 0
    for f in nc.m.functions:
        for bb in f.blocks:
            insts = bb.instructions
            out = []
            for inst in insts:
                si = inst.sync_info
                waits = list(si.on_wait) if si is not None and si.on_wait else []
                if len(waits) > max_waits:
                    extra = waits[: len(waits) - max_waits]
                    keep = waits[len(waits) - max_waits:]
                    while extra:
                        chunk, extra = extra[:max_waits], extra[max_waits:]
                        nop = InstNoOp(name=f"I-waitfix-{ctr}", ins=[], outs=[])
                        ctr += 1
                        nop.engine = inst.engine
                        nop.sync_info = mybir.SyncInfo(on_wait=chunk, on_update=[])
                        out.append(nop)
                    si.on_wait = keep
                    inst.sync_info = si
                out.append(inst)
            if len(out) != len(insts):
                insts[:] = out
    return ctr


# ------------------------------------------------- custom fused DVE ops
def _register_ops():
    """RELU_ADD_REDUCE: out = relu(in0+in1)*imm2, accum = s0 + sum(out).
    AFFINE_NORM_SCALE: out = (in0 - s0) * s1 * in1."""
    from operator import add as _add
    import concourse.dve_ops as dve_ops
    from concourse.dve_ops import DveOp
    from concourse.dve_spec import C0, C1, C2, Spec, Src0, Src1, relu
    from concourse.dve_spec import lower as spec_lower
    from concourse.dve_uop import DveOpSpec

    def mk(name, spec):
        for op in dve_ops.OPS:
            if op.name == name:
                return op
        shas = {}
        for ver in ("v3", "v4"):
            try:
                compiled = DveOpSpec(
                    name=name, opcode=0, uops=spec_lower(spec, ver=ver),
                    rd1_en=True)
                shas[ver] = compiled.sha(ver)
            except Exception:
                pass
        op = DveOp(name, spec, subdim=False, uops_sha=shas)
        dve_ops.OPS.append(op)
        dve_ops.CUSTOM_DVE_SPECS[op.name] = op.spec
        dve_ops._SUB_OPCODE_FOR_NAME[op.name] = (
            dve_ops._CUSTOM_DVE_ROW_BASE + len(dve_ops.OPS) - 1)
        assert dve_ops._SUB_OPCODE_FOR_NAME[op.name] < 0x20
        return op

    def _ref_rar(in0, in1, s0, s1, imm2):
        b = (np.maximum(in0.astype(np.float32) + in1, 0) * imm2).astype(np.float32)
        acc = np.asarray(s0, np.float32).reshape(-1, 1) + b.reshape(
            b.shape[0], -1).sum(-1, keepdims=True)
        return b, acc

    rar = mk("RELU_ADD_REDUCE_ANT", Spec(
        body=relu(Src0 + Src1) * C2, accum=_add, accum_init=C0,
        reference=_ref_rar))

    afn = mk("AFFINE_NORM_SCALE_ANT", Spec(
        body=(Src0 - C0) * C1 * Src1,
        reference=lambda in0, in1, s0, s1, imm2: (
            (in0.astype(np.float32) - s0) * s1 * in1)))
    return rar, afn


# ------------------------------------------------------------- host helpers
def _wrap_idx16(vals):
    """dma_gather index layout: idx j lives at [j%16, j//16], replicated to
    128 partitions."""
    n = len(vals)
    assert n % 16 == 0
    a = np.asarray(vals, np.int16).reshape(-1, 16).T.copy()  # [16, n//16]
    return np.tile(a, (8, 1))


def calls_of(K):
    out = []
    c = 0
    while c < K:
        out.append(min(MAXSLOT, K - c))
        c += MAXSLOT
    return out


def _build_grids(owned_nodes, adj_rows, other_val, K_sched):
    """Dense [node x slot] grid for one grouping on one core.

    other_val: per-edge value to store in the cell (e.g. other-endpoint id,
    or edge id).  Returns (vals [128, C] int64, mask [128, C] f32,
    emap (eid, p, col) triplets) -- emap only if other_val is edge id? No:
    emap always lists (eid, p, col)."""
    C = int(sum(K_sched))
    mask = np.zeros((128, C), np.float32)
    vals = np.zeros((128, C), np.int64)
    emap = []
    col0 = 0
    for t in range(NT):
        K = int(K_sched[t])
        for p in range(128):
            n = owned_nodes[t * 128 + p]
            if n < 0:
                continue
            edges = adj_rows.get(n, ())
            assert len(edges) <= K
            for c, eid in enumerate(edges):
                mask[p, col0 + c] = 1.0
                vals[p, col0 + c] = other_val[eid]
                emap.append((eid, p, col0 + c))
        col0 += K
    return vals, mask, emap


def _kernel_cached():
    if not hasattr(_kernel_cached, "ops"):
        _kernel_cached.ops = _register_ops()
    return _kernel_cached.ops


def kernel(node_features, edge_index, num_nodes, W1, b1, g1, bb1,
           W2, b2, g2, bb2, W3, b3, W4, b4):
    node_features = np.asarray(node_features, np.float32)
    edge_index = np.asarray(edge_index).astype(np.int64)
    W1 = np.asarray(W1, np.float32); W2 = np.asarray(W2, np.float32)
    b1 = np.asarray(b1, np.float32); b2 = np.asarray(b2, np.float32)
    g1 = np.asarray(g1, np.float32); g2 = np.asarray(g2, np.float32)
    bb1 = np.asarray(bb1, np.float32); bb2 = np.asarray(bb2, np.float32)
    W3 = np.asarray(W3, np.float32); b4f = float(np.asarray(b4).reshape(-1)[0])
    w4f = float(np.asarray(W4).reshape(-1)[0])
    assert int(num_nodes) == N
    assert node_features.shape == (B, N, F) and edge_index.shape == (B, 2, E)
    assert np.all(b1 == 0) and np.all(b2 == 0), "nonzero b1/b2 unsupported"
    assert np.all(bb1 == 0) and np.all(bb2 == 0), "nonzero bb1/bb2 unsupported"

    rar_op, afn_op = _kernel_cached()

    w3 = W3[:, 0]
    sigma = np.argsort(w3 < 0, kind="stable")   # nonneg cols first
    posl = int((w3 >= 0).sum())
    nneg = 512 - posl
    G1 = (g1 * w3)[sigma].astype(np.float32)
    G2 = (g2 * w3)[sigma].astype(np.float32)
    W1p = W1[:, sigma]; W2p = W2[:, sigma]

    # ---------------- host sharding / grids
    srcs = edge_index[:, 0, :]; dsts = edge_index[:, 1, :]
    quarter = np.minimum(np.arange(N) // (N // NQ), NQ - 1)

    core_meta = []
    Ks_ij = np.zeros(NT, np.int64); Ks_ji = np.zeros(NT, np.int64)
    for b in range(B):
        s, t = srcs[b], dsts[b]
        outdeg = np.bincount(s, minlength=N)
        indeg = np.bincount(t, minlength=N)
        out_adj = {}; in_adj = {}
        order = np.argsort(s, kind="stable")
        bounds = np.searchsorted(s[order], np.arange(N + 1))
        for n in range(N):
            lo, hi = bounds[n], bounds[n + 1]
            if hi > lo:
                out_adj[n] = order[lo:hi]
        order2 = np.argsort(t, kind="stable")
        bounds2 = np.searchsorted(t[order2], np.arange(N + 1))
        for n in range(N):
            lo, hi = bounds2[n], bounds2[n + 1]
            if hi > lo:
                in_adj[n] = order2[lo:hi]
        for q in range(NQ):
            nodes = np.where(quarter == q)[0]
            o_ij = nodes[np.argsort(-outdeg[nodes], kind="stable")]
            o_ji = nodes[np.argsort(-indeg[nodes], kind="stable")]
            own_ij = np.full(NSH, -1, np.int64); own_ij[:len(o_ij)] = o_ij
            own_ji = np.full(NSH, -1, np.int64); own_ji[:len(o_ji)] = o_ji
            for tt in range(NT):
                seg = own_ij[tt * 128:(tt + 1) * 128]
                deg = outdeg[seg[seg >= 0]]
                Ks_ij[tt] = max(Ks_ij[tt], deg.max() if len(deg) else 0)
                seg = own_ji[tt * 128:(tt + 1) * 128]
                deg = indeg[seg[seg >= 0]]
                Ks_ji[tt] = max(Ks_ji[tt], deg.max() if len(deg) else 0)
            core_meta.append(dict(b=b, q=q, own_ij=own_ij, own_ji=own_ji,
                                  out_adj=out_adj, in_adj=in_adj))
    Ks_ij = np.maximum(Ks_ij, 1); Ks_ji = np.maximum(Ks_ji, 1)
    C_ij = int(Ks_ij.sum()); C_ji = int(Ks_ji.sum())

    CHROWS = NSH // NCHUNK

    # global Y row for node n per batch (chunked allgather layout)
    yrow = np.zeros((B, N), np.int64)
    for cm in core_meta:
        b, q = cm["b"], cm["q"]
        nodes = cm["own_ij"][cm["own_ij"] >= 0]
        l = np.arange(len(nodes))
        yrow[b, nodes] = ((l // CHROWS) * (8 * CHROWS)
                         + (b * NQ + q) * CHROWS + (l % CHROWS))

    nfT = node_features.transpose(0, 2, 1)  # [B, F, N]

    # First build all ij grids (need edge->grid-position for the d-table).
    ij_grids = []
    for ci, cm in enumerate(core_meta):
        b, q = cm["b"], cm["q"]
        rows_ij, mask_ij, emap_ij = _build_grids(
            cm["own_ij"], cm["out_adj"], dsts[b], Ks_ij)
        ij_grids.append((rows_ij, mask_ij, emap_ij))

    # d-table flat index per edge: F = (q*128 + p)*C_ij + col
    Fidx = np.zeros((B, E), np.int64)
    for ci, cm in enumerate(core_meta):
        b, q = cm["b"], cm["q"]
        _, _, emap_ij = ij_grids[ci]
        if emap_ij:
            eid, p, col = np.array(emap_ij).T
            Fidx[b, eid] = ((b * NQ + q) * 128 + p) * C_ij + col

    per_core_inputs = []
    per_core_maps = []
    for ci, cm in enumerate(core_meta):
        b, q = cm["b"], cm["q"]
        rows_ij, mask_ij, emap_ij = ij_grids[ci]
        gy_ij = yrow[b][rows_ij]          # [128, C_ij] global Y rows

        def idx_stream(gy, Ks):
            words = []
            col0 = 0
            for tt in range(NT):
                for ns in calls_of(int(Ks[tt])):
                    blk = gy[:, col0:col0 + ns]          # [128, ns]
                    vals = blk.T.reshape(-1)             # j = c*128 + p
                    words.append(_wrap_idx16(vals))
                    col0 += ns
            return np.concatenate(words, axis=1)
        idx_ij = idx_stream(gy_ij, Ks_ij)

        # ji grid: cells hold edge ids; gather the d-table rows F//128 and
        # one-hot select lane F%128.
        fgrid, mask_ji, emap_ji = _build_grids(
            cm["own_ji"], cm["in_adj"], Fidx[b], Ks_ji)
        drow = fgrid // 128                 # [128, C_ji]
        lane = fgrid % 128
        idx_d = idx_stream(drow, Ks_ji)
        selmask = np.full((128, C_ji, 128), -BIGS, np.float32)
        P, CC = np.nonzero(mask_ji)
        selmask[P, CC, lane[P, CC]] = BIGS
        selmask = selmask.astype(ml_dtypes.bfloat16)

        # phase-1 inputs
        nf_sl = np.zeros((F, NSH), np.float32)
        nodes = cm["own_ij"][cm["own_ij"] >= 0]
        nf_sl[:, :len(nodes)] = nfT[b][:, nodes]
        nfT_in = np.ascontiguousarray(
            nf_sl.reshape(4, 128, NSH).transpose(1, 0, 2)).astype(
                ml_dtypes.bfloat16)
        Win = np.stack([W1p, W2p], 0)     # [2, F, H]
        W_in = np.ascontiguousarray(
            Win.transpose(1, 0, 2).reshape(4, 128, 2, H).transpose(
                1, 0, 2, 3)).astype(ml_dtypes.bfloat16)  # [128,4,2,H]
        wsum = np.stack([W1p.sum(1), W2p.sum(1)], 1)  # [F, 2]
        wsum_in = np.ascontiguousarray(
            wsum.reshape(4, 128, 2).transpose(1, 0, 2)).astype(
                ml_dtypes.bfloat16)
        G_in = np.tile((SCL * np.concatenate([G1, G2]))[None, :],
                       (128, 1)).astype(np.float32)  # [128, 1024]
        per_core_inputs.append({
            "nfT": nfT_in, "W": W_in, "wsum": wsum_in, "G": G_in,
            "idx_ij": idx_ij.astype(np.int16), "idx_d": idx_d.astype(np.int16),
            "mask_ij": mask_ij, "mask_ji": mask_ji, "selmask": selmask,
        })
        per_core_maps.append((emap_ij, emap_ji))

    IW_ij = per_core_inputs[0]["idx_ij"].shape[1]
    IW_d = per_core_inputs[0]["idx_d"].shape[1]
    for pci in per_core_inputs:
        assert pci["idx_ij"].shape[1] == IW_ij
        assert pci["idx_d"].shape[1] == IW_d

    # ---------------------------------------------------------------- device
    nc = _build_program(rar_op, afn_op, posl, nneg, w4f, b4f,
                        IW_ij, IW_d, C_ij, C_ji, Ks_ij, Ks_ji)

    import os
    trace = bool(os.environ.get("KERNEL_TRACE"))
    res = run_bass_kernel_spmd(nc, per_core_inputs, core_ids=list(range(8)),
                               trace=trace)
    kernel.last_result = res

    # ------------------------------------------------------------ assemble
    Vij = np.zeros((B, E), np.float32)
    Vji = np.zeros((B, E), np.float32)
    for ci in range(8):
        b = core_meta[ci]["b"]
        out_ij = res.results[ci]["out_ij"]
        out_ji = res.results[ci]["out_ji"]
        emap_ij, emap_ji = per_core_maps[ci]
        if emap_ij:
            eid, p, col = np.array(emap_ij).T
            Vij[b, eid] = out_ij[p, col]
        if emap_ji:
            eid, p, col = np.array(emap_ji).T
            Vji[b, eid] = out_ji[p, col]
    return Vij, Vji


def _build_program(rar_op, afn_op, posl, nneg, w4f, b4f,
                   IW_ij, IW_d, C_ij, C_ji, Ks_ij, Ks_ji):
    nc = bass.Bass(num_devices=8)
    nfT = nc.dram_tensor("nfT", [128, 4, NSH], bf16, kind="ExternalInput")
    W = nc.dram_tensor("W", [128, 4, 2, H], bf16, kind="ExternalInput")
    wsum = nc.dram_tensor("wsum", [128, 4, 2], bf16, kind="ExternalInput")
    G = nc.dram_tensor("G", [128, 2 * H], f32, kind="ExternalInput")
    idx_ij = nc.dram_tensor("idx_ij", [128, IW_ij], mybir.dt.int16,
                            kind="ExternalInput")
    idx_d = nc.dram_tensor("idx_d", [128, IW_d], mybir.dt.int16,
                           kind="ExternalInput")
    mask_ij = nc.dram_tensor("mask_ij", [128, C_ij], f32, kind="ExternalInput")
    mask_ji = nc.dram_tensor("mask_ji", [128, C_ji], f32, kind="ExternalInput")
    selmask = nc.dram_tensor("selmask", [128, C_ji, 128], bf16,
                             kind="ExternalInput")
    out_ij = nc.dram_tensor("out_ij", [128, C_ij], f32, kind="ExternalOutput")
    out_ji = nc.dram_tensor("out_ji", [128, C_ji], f32, kind="ExternalOutput")
    Ysh = nc.dram_tensor("Ysh", [NSH, ROW], f8)
    CHROWS = NSH // NCHUNK
    Yfull = nc.dram_tensor("Yfull", [2 * NQ * NSH, ROW], f8, addr_space="Shared")
    Dsh = nc.dram_tensor("Dsh", [128, C_ij], bf16)
    Dall = nc.dram_tensor("Dall", [2 * NQ * C_ij, 128], bf16, addr_space="Shared")
    KSMAX = int(max(max(Ks_ij), max(Ks_ji)) + 2)

    with tile.TileContext(nc) as tc:
        with tc.tile_pool(name="persist", bufs=1) as pp:
            res1 = pp.tile([128, NT, 2 * BW], bf16)   # [PLUS | MINUS] rows
            delta1 = pp.tile([128, NT], f32)
            Gt = pp.tile([128, 2 * H], f32)
            dg = pp.tile([128, C_ij], f32)
            sji = pp.tile([128, C_ji], f32)
            oij = pp.tile([128, C_ij], f32)
            oji = pp.tile([128, C_ji], f32)
            nc.sync.dma_start(out=Gt[:], in_=G[:])
            cbias = pp.tile([128, 4], f32)   # eps | b4 | -40 | b4+w4*BIGS
            nc.vector.memset(cbias[:, 0:1], EPS)
            nc.vector.memset(cbias[:, 1:2], b4f)
            nc.vector.memset(cbias[:, 2:3], -40.0)
            nc.vector.memset(cbias[:, 3:4], b4f + (w4f / SCL) * BIGS)
            ztail = pp.tile([128, ROW - 2 * BW - 1], f8)
            nc.vector.memset(ztail[:], 0.0)
            tailb = pp.tile([128, NT], f8)
            nc.gpsimd.load_library(library_config.mlp)

            # ---------------- phase 1 ----------------
            with tc.tile_pool(name="p1", bufs=1) as p1, \
                 tc.tile_pool(name="p1b", bufs=4) as p1b, \
                 tc.tile_pool(name="ps", bufs=2, space="PSUM") as ps, \
                 tc.tile_pool(name="ps2", bufs=2, space="PSUM") as ps2:
                nft = p1.tile([128, 4, NSH], bf16)
                Wt = p1.tile([128, 4, 2, H], bf16)
                wst = p1.tile([128, 4, 2], bf16)
                nc.sync.dma_start(out=nft[:], in_=nfT[:])
                nc.sync.dma_start(out=Wt[:], in_=W[:])
                nc.sync.dma_start(out=wst[:], in_=wsum[:])

                for t in range(NT):
                    stats = ps2.tile([128, 2], f32, tag="stats")
                    um = []
                    for m in range(2):
                        u = ps.tile([128, H], f32, tag=f"u{m}")
                        um.append(u)
                    for fc in range(4):
                        lhsT = nft[:, fc, t * 128:(t + 1) * 128]
                        for m in range(2):
                            nc.tensor.matmul(
                                um[m][:], lhsT, Wt[:, fc, m, :],
                                start=(fc == 0), stop=(fc == 3))
                        nc.tensor.matmul(
                            stats[:], lhsT, wst[:, fc, :],
                            start=(fc == 0), stop=(fc == 3))
                    for m in range(2):
                        sq = p1b.tile([128, H], bf16, tag="sq")
                        s2 = p1b.tile([128, 1], f32, tag="s2")
                        nc.scalar.activation(
                            out=sq[:], in_=um[m][:],
                            func=mybir.ActivationFunctionType.Square,
                            accum_out=s2[:, 0:1])
                        mean = p1b.tile([128, 1], f32, tag="mean")
                        nc.vector.tensor_scalar_mul(
                            out=mean[:], in0=stats[:, m:m + 1], scalar1=1.0 / H)
                        m2 = p1b.tile([128, 1], f32, tag="m2")
                        nc.vector.tensor_tensor(
                            out=m2[:], in0=mean[:], in1=mean[:],
                            op=mybir.AluOpType.mult)
                        var = p1b.tile([128, 1], f32, tag="var")
                        nc.vector.tensor_scalar(
                            out=var[:], in0=s2[:], scalar1=1.0 / H,
                            scalar2=m2[:, 0:1], op0=mybir.AluOpType.mult,
                            op1=mybir.AluOpType.subtract)
                        sd = p1b.tile([128, 1], f32, tag="sd")
                        nc.scalar.activation(
                            out=sd[:], in_=var[:],
                            func=mybir.ActivationFunctionType.Sqrt,
                            bias=cbias[:, 0:1])
                        rstd = p1b.tile([128, 1], f32, tag="rstd")
                        nc.vector.reciprocal(out=rstd[:], in_=sd[:])
                        # res1 layout: [X1p(0:posl), X2n(posl:512) |
                        #               X2p(512:512+posl), X1n(512+posl:1024)]
                        if m == 0:
                            po, no = 0, BW + posl
                        else:
                            po, no = BW, posl
                        nc.vector._custom_dve(
                            afn_op, out=res1[:, t, po:po + posl],
                            in0=um[m][:, 0:posl],
                            in1=Gt[:, m * H:m * H + posl],
                            s0=mean[:, 0:1], s1=rstd[:, 0:1])
                        nc.vector._custom_dve(
                            afn_op, out=res1[:, t, no:no + nneg],
                            in0=um[m][:, posl:512],
                            in1=Gt[:, m * H + posl:m * H + 512],
                            s0=mean[:, 0:1], s1=rstd[:, 0:1])
                    # delta = sum(X1n) - sum(X2n)
                    dscr = p1b.tile([128, 512], bf16, tag="dscr")
                    nc.vector.scalar_tensor_tensor(
                        out=dscr[:, 0:nneg],
                        in0=res1[:, t, BW + posl:BW + posl + nneg], scalar=0.0,
                        in1=res1[:, t, posl:posl + nneg],
                        op0=mybir.AluOpType.bypass,
                        op1=mybir.AluOpType.subtract,
                        accum_out=delta1[:, t:t + 1])
                    nc.vector.tensor_copy(
                        out=tailb[:, t:t + 1], in_=delta1[:, t:t + 1])
                    # Y row = [res MINUS | res PLUS | delta | zeros]  (fp8)
                    yst = p1b.tile([128, 2 * BW], f8, tag="yst")
                    nc.scalar.activation(
                        out=yst[:, 0:BW], in_=res1[:, t, BW:2 * BW],
                        func=mybir.ActivationFunctionType.Copy)
                    nc.scalar.activation(
                        out=yst[:, BW:2 * BW], in_=res1[:, t, 0:BW],
                        func=mybir.ActivationFunctionType.Copy)
                    ysl = Ysh.rearrange("(a p) c -> p a c", p=128)
                    nc.sync.dma_start(out=ysl[:, t, 0:2 * BW], in_=yst[:])
                    nc.sync.dma_start(
                        out=ysl[:, t, 2 * BW:2 * BW + 1], in_=tailb[:, t:t + 1])
                    nc.sync.dma_start(
                        out=ysl[:, t, 2 * BW + 1:ROW], in_=ztail[:])
                    tpc = NT // NCHUNK
                    if (t + 1) % tpc == 0:
                        ch = (t + 1) // tpc - 1
                        nc.gpsimd.collective_compute(
                            "AllGather", mybir.AluOpType.bypass,
                            replica_groups=[list(range(8))],
                            ins=[Ysh[ch * CHROWS:(ch + 1) * CHROWS, :].opt()],
                            outs=[Yfull[ch * 8 * CHROWS:(ch + 1) * 8 * CHROWS,
                                        :].opt()])

            # ---------------- edge pass (single, by src) ----------------
            nidx_regs = {}

            def nidx_reg(n):
                if n not in nidx_regs:
                    nidx_regs[n] = nc.gpsimd.to_reg(n)
                return nidx_regs[n]

            def softmax_stagemajor(vals, maskt, out_t, tile_cols, scale,
                                   bias_col, sbp):
                # stage-major over all tiles so V/ACT ping-pong pipelines
                n = len(tile_cols)
                vv, evv, rss = [], [], []
                for i, (cl, cr) in enumerate(tile_cols):
                    K = cr - cl
                    v = sbp.tile([128, KSMAX], f32, tag=f"v{i}")
                    nc.scalar.activation(
                        out=v[:, 0:K], in_=vals[:, cl:cr],
                        func=mybir.ActivationFunctionType.Relu,
                        bias=cbias[:, bias_col:bias_col + 1], scale=scale)
                    vv.append(v)
                for i, (cl, cr) in enumerate(tile_cols):
                    K = cr - cl
                    nc.vector.scalar_tensor_tensor(
                        out=vv[i][:, 0:K], in0=vv[i][:, 0:K], scalar=40.0,
                        in1=maskt[:, cl:cr], op0=mybir.AluOpType.add,
                        op1=mybir.AluOpType.mult)
                for i, (cl, cr) in enumerate(tile_cols):
                    K = cr - cl
                    ssum = sbp.tile([128, 1], f32, tag=f"ss{i}")
                    ev = sbp.tile([128, KSMAX], f32, tag=f"ev{i}")
                    nc.scalar.activation(
                        out=ev[:, 0:K], in_=vv[i][:, 0:K],
                        func=mybir.ActivationFunctionType.Exp,
                        bias=cbias[:, 2:3], accum_out=ssum[:, 0:1])
                    evv.append(ev); rss.append(ssum)
                for i in range(n):
                    nc.vector.reciprocal(out=rss[i][:], in_=rss[i][:])
                for i, (cl, cr) in enumerate(tile_cols):
                    K = cr - cl
                    nc.vector.tensor_scalar_mul(
                        out=out_t[:, cl:cr], in0=evv[i][:, 0:K],
                        scalar1=rss[i][:, 0:1])

            with tc.tile_pool(name="ep", bufs=1) as ep, \
                 tc.tile_pool(name="gb", bufs=4) as gb, \
                 tc.tile_pool(name="sb", bufs=2) as sbp:
                idxt = ep.tile([128, IW_ij], mybir.dt.int16)
                maskt = ep.tile([128, C_ij], f32)
                nc.sync.dma_start(out=idxt[:], in_=idx_ij[:])
                nc.sync.dma_start(out=maskt[:], in_=mask_ij[:])
                iw = 0
                col0 = 0
                tile_cols = []
                for t in range(NT):
                    for ns in calls_of(int(Ks_ij[t])):
                        g = gb.tile([128, MAXSLOT, ROW], f8, tag="g")
                        nidx = ns * 128
                        nc.gpsimd.dma_gather(
                            g[:, 0:ns, :], Yfull[:],
                            idxt[:, iw:iw + nidx // 16],
                            nidx, nidx_reg(nidx), ROW)
                        iw += nidx // 16
                        for c in range(ns):
                            col = col0 + c
                            acc = dg[:, col:col + 1]
                            scr = sbp.tile([128, BW], bf16, tag="scr0")
                            nc.vector._custom_dve(
                                rar_op, out=scr[:],
                                in0=res1[:, t, 0:BW], in1=g[:, c, 0:BW],
                                s0=0.0, imm2=1.0, accum_out=acc)
                            scr2 = sbp.tile([128, BW], bf16, tag="scr1")
                            nc.vector._custom_dve(
                                rar_op, out=scr2[:],
                                in0=res1[:, t, BW:2 * BW],
                                in1=g[:, c, BW:2 * BW],
                                s0=acc, imm2=-1.0, accum_out=acc)
                        # d -= delta(j)  (delta col of the gathered rows)
                        nc.vector.tensor_tensor(
                            out=dg[:, col0:col0 + ns],
                            in0=dg[:, col0:col0 + ns],
                            in1=g[:, 0:ns, 2 * BW],
                            op=mybir.AluOpType.subtract)
                        col0 += ns
                    # d += delta(i)
                    K = int(Ks_ij[t])
                    cl, cr = col0 - K, col0
                    nc.vector.tensor_scalar(
                        out=dg[:, cl:cr], in0=dg[:, cl:cr],
                        scalar1=delta1[:, t:t + 1], scalar2=None,
                        op0=mybir.AluOpType.add,
                        op1=mybir.AluOpType.bypass)
                    tile_cols.append((cl, cr))
                # ship d (=SCL*d, bf16) to the group, then overlap the
                # collective with the deferred softmaxes
                dsb = ep.tile([128, C_ij], bf16)
                nc.vector.tensor_copy(out=dsb[:], in_=dg[:])
                nc.sync.dma_start(out=Dsh[:], in_=dsb[:])
                nc.gpsimd.collective_compute(
                    "AllGather", mybir.AluOpType.bypass,
                    replica_groups=[list(range(8))],
                    ins=[Dsh[:].opt()], outs=[Dall[:].opt()])
                softmax_stagemajor(dg, maskt, oij, tile_cols, w4f / SCL, 1,
                                   sbp)
            nc.sync.dma_start(out=out_ij[:], in_=oij[:])

            # ---------------- ji pass: gather d scalars, softmax by dst ----
            with tc.tile_pool(name="ep2", bufs=1) as ep2, \
                 tc.tile_pool(name="gb2", bufs=4) as gb2, \
                 tc.tile_pool(name="sm", bufs=2) as smp, \
                 tc.tile_pool(name="sb2", bufs=2) as sbp2:
                idxt2 = ep2.tile([128, IW_d], mybir.dt.int16)
                maskt2 = ep2.tile([128, C_ji], f32)
                nc.sync.dma_start(out=idxt2[:], in_=idx_d[:])
                nc.sync.dma_start(out=maskt2[:], in_=mask_ji[:])
                iw = 0
                col0 = 0
                tile_cols2 = []
                for t in range(NT):
                    K = int(Ks_ji[t])
                    tc0 = col0
                    smt = smp.tile([128, KSMAX, 128], bf16, tag="smt")
                    nc.sync.dma_start(
                        out=smt[:, 0:K, :], in_=selmask[:, tc0:tc0 + K, :])
                    for ns in calls_of(K):
                        g2 = gb2.tile([128, MAXSLOT, 128], bf16, tag="g2")
                        nidx = ns * 128
                        nc.gpsimd.dma_gather(
                            g2[:, 0:ns, :], Dall[:],
                            idxt2[:, iw:iw + nidx // 16],
                            nidx, nidx_reg(nidx), 128)
                        iw += nidx // 16
                        for c in range(ns):
                            col = col0 + c
                            scr = sbp2.tile([128, 128], bf16, tag="scr")
                            nc.vector._custom_dve(
                                rar_op, out=scr[:],
                                in0=g2[:, c, :],
                                in1=smt[:, col - tc0, :],
                                s0=0.0, imm2=1.0,
                                accum_out=sji[:, col:col + 1])
                        col0 += ns
                    tile_cols2.append((tc0, tc0 + K))
                softmax_stagemajor(sji, maskt2, oji, tile_cols2, -w4f / SCL,
                                   3, sbp2)
            nc.sync.dma_start(out=out_ji[:], in_=oji[:])

    mybir.codegen_inst_isa_subclasses(nc)
    _split_waits(nc)
    return nc
